# revision 6
# baseline (speedup 1.0000x reference)
"""Trainium2 Bass kernel for nn_BitResidualBlock (dense_cnn).

Reference computation (per batch element, C=512 channels, T=4096):
    for d in (1, 3, 5):
        h = bitconv1d(x, w1, b1, dilation=d)     # ternary-quantized weights
        h = snake_beta(h, alpha, beta)           # x + sin(a*x)^2 / (b+eps)
        h = bitconv1d(h, w2, b2, dilation=1)
        x = x + h

Strategy (v2, Winograd):
  - Data-parallel over batch: 8 batch elements -> 8 NeuronCores.
  - All 6 convs use Winograd F(2,3) over the dilation-d lattice: per
    output pair (y[t], y[t+d]) four transform points are matmul'd with
    transformed ternary weights U = G w (values {0,+-.5,+-1,+-1.5},
    exact in bf16). Tensor-engine columns drop 3T -> 2T (1.5x).
  - Activations live in even/odd phase planes (bf16) so the B^T input
    transform is built with dense/strided scalar_tensor_tensor ops that
    hit the DVE 2x/4x fast paths.
  - The 4 transform-point matmuls accumulate into 4 PSUM banks of one
    [128, 2048] tile; a single ACT drain applies the absmean scale s.
  - A^T + bias fold into 2-op stt chains; snake uses mod-based range
    reduction (sin^2 is pi-periodic, so r - int(r) with any rounding
    works); residual accumulates in bf16 planes.
"""

import numpy as np
import ml_dtypes

import concourse.bass as bass
import concourse.mybir as mybir
import concourse.tile as tile
from concourse.vector_clock import ScopedClock
from concourse.bass_utils import run_bass_kernel_spmd


def _ceil_div(a, b):
    return -(-a // b)


# F(2,3) streams: (delta0, delta1, op) with v = x[t0+d*delta0] op x[t0+d*delta1]
STREAMS = [(-1, 1, "sub"), (0, 1, "add"), (1, 0, "sub"), (0, 2, "sub")]


def geom(d, T=4096):
    nblk = _ceil_div(T, 2 * d)
    NL = nblk * d
    units = []
    S = 0
    while S < NL:
        w = min(512, NL - S)
        units.append((S, w))
        S += w
    return NL, units


def _src_spec(d, q, delta, jlo, cnt):
    par = (q + delta) & 1
    col0 = jlo * d + (q + delta - par) // 2
    return (par, col0, d, cnt)


def v_ops(d, S, w):
    for a, (e0, e1, op) in enumerate(STREAMS):
        d0, d1 = e0 * d, e1 * d
        for q in range(d):
            jlo = _ceil_div(S - q, d)
            jhi = _ceil_div(S + w - q, d)
            cnt = jhi - jlo
            if cnt <= 0:
                continue
            out_off = jlo * d + q - S
            yield (a, out_off, d, cnt,
                   _src_spec(d, q, d0, jlo, cnt),
                   _src_spec(d, q, d1, jlo, cnt), op)


def h_ops(d, S, w, T=4096):
    for hf in (0, 1):
        for q in range(d):
            jlo = _ceil_div(S - q, d)
            jhi = _ceil_div(S + w - q, d)
            jhi = min(jhi, _ceil_div(T - q - hf * d, 2 * d))
            cnt = jhi - jlo
            if cnt <= 0:
                continue
            tile_off = jlo * d + q - S
            par = (q + hf * d) & 1
            pcol0 = jlo * d + (q + hf * d - par) // 2
            yield (hf, tile_off, d, cnt, par, pcol0)

AF = mybir.ActivationFunctionType
ALU = mybir.AluOpType
F32 = mybir.dt.float32
I32 = mybir.dt.int32
BF16 = mybir.dt.bfloat16

B, C, T, K = 8, 512, 4096, 3
DILATIONS = (1, 3, 5)
EPS_Q = 1e-5
EPS_SNAKE = 1e-9

P = 128          # partitions
NCH = C // P     # 4 channel chunks
NE = T // 2      # 2048 columns per phase plane
PAD = 8
PW = PAD + NE + PAD
NPARAM = 18      # 6 param columns per block x 3 blocks
USE_MOD = False  # mod-ALU rejected by walrus ISA check; i32 convert fallback

# Set by the test harness to profile; kernel() records exec time here.
TRACE = False
LAST_EXEC_NS = None
LAST_RESULT = None


class SplitDrainTileContext(tile.TileContext):
    """TileContext whose tail drain splits its sem waits across
    single-wait instructions (walrus rejects multi-wait Drain)."""

    def _drain_and_barrier(self, tick_clock, wait_clock):
        collector = self.nc.sync.nop(nofuse=True)
        wait_clock.add_sem_waits(
            collector.ins, ScopedClock({None: tick_clock.global_clock})
        )
        si = collector.ins.sync_info
        waits = list(si.on_wait) if si is not None else []
        if len(waits) > 1:
            collector.ins.sync_info = mybir.SyncInfo(
                on_wait=waits[:1], on_update=list(si.on_update)
            )
            for w in waits[1:]:
                extra = self.nc.sync.nop(nofuse=True)
                extra.ins.sync_info = mybir.SyncInfo(on_wait=[w], on_update=[])
        self.nc.sync.drain()
        self.nc.all_engine_barrier()
        assert self.sems is not None
        popped = self.nc._tile_sem_poison_stack.pop()
        assert popped is self._sem_poison
        self.nc.clear_and_free_semaphores(list(self.sems.allocated().values()))
        self.nc.all_engine_barrier()


def _split_sync_waits(nc, maxw=1):
    """Move excess sync waits onto single-wait EventSemaphore instructions
    inserted before the owner on the same engine."""
    for bb in nc.main_func.blocks:
        out = []
        changed = False
        for ins in bb.instructions:
            si = getattr(ins, "sync_info", None)
            if si is not None and len(si.on_wait) > maxw:
                waits = list(si.on_wait)
                extra, keep = waits[:-maxw], waits[-maxw:]
                for w in extra:
                    ev = mybir.InstEventSemaphore(
                        name=nc.get_next_instruction_name(), ins=[], outs=[])
                    ev.engine = ins.engine
                    ev.sync_info = mybir.SyncInfo(on_wait=[w], on_update=[])
                    nc.register_instruction(ev, overwrite=True)
                    out.append(ev)
                ins.sync_info = mybir.SyncInfo(
                    on_wait=keep, on_update=list(si.on_update))
                changed = True
            out.append(ins)
        if changed:
            bb.instructions = out


def _ap_strided(t, col0, step, cnt, base=PAD):
    """AP on plane tile t: cols base+col0, stride step, count cnt."""
    c0 = base + col0
    if step == 1:
        return t[:, c0:c0 + cnt]
    return t[:, c0:c0 + (cnt - 1) * step + 1:step]


def build_nc():
    nc = bass.Bass(target_bir_lowering=False)
    xE_d = nc.dram_tensor("xE", [C, NE], BF16, kind="ExternalInput")
    xO_d = nc.dram_tensor("xO", [C, NE], BF16, kind="ExternalInput")
    u_d = nc.dram_tensor("u", [6, 4, NCH, P, C], BF16, kind="ExternalInput")
    pp_d = nc.dram_tensor("pp", [NCH, P, NPARAM], F32, kind="ExternalInput")
    y_d = nc.dram_tensor("y", [C, T], F32, kind="ExternalOutput")

    with SplitDrainTileContext(nc) as tc:
        with (
            tc.tile_pool(name="persist", bufs=1) as p1,
            tc.tile_pool(name="wts", bufs=1) as pw,
            tc.tile_pool(name="vv", bufs=2) as pv,
            tc.tile_pool(name="mq", bufs=2) as pm,
            tc.tile_pool(name="tmp", bufs=2) as ptm,
            tc.tile_pool(name="snk", bufs=2) as psn,
            tc.tile_pool(name="yo", bufs=2) as py,
            tc.tile_pool(name="ps", bufs=2, space="PSUM") as pps,
        ):
            xpl = [[p1.tile([P, PW], BF16, tag=f"x{pl}{c}", name=f"x{pl}{c}")
                    for c in range(NCH)] for pl in range(2)]
            hpl = [[p1.tile([P, PW], BF16, tag=f"h{pl}{c}", name=f"h{pl}{c}")
                    for c in range(NCH)] for pl in range(2)]
            pt = [p1.tile([P, NPARAM], F32, tag=f"pt{c}", name=f"pt{c}")
                  for c in range(NCH)]

            for c in range(NCH):
                nc.sync.dma_start(out=pt[c], in_=pp_d[c])
            for pl in range(2):
                for c in range(NCH):
                    for tt in (xpl, hpl):
                        nc.vector.memset(tt[pl][c][:, 0:PAD], 0.0)
                        nc.vector.memset(tt[pl][c][:, PAD + NE:PW], 0.0)

            def load_u(cidx):
                ut = [[pw.tile([P, C], BF16, tag=f"u{cidx % 2}_{a}_{ci}",
                               name=f"u{cidx}_{a}_{ci}")
                       for ci in range(NCH)] for a in range(4)]
                for a in range(4):
                    for ci in range(NCH):
                        nc.sync.dma_start(out=ut[a][ci], in_=u_d[cidx, a, ci])
                return ut

            # DMA order: params, U0, x quarters, U1 (rest later)
            u_tiles = {0: load_u(0)}
            for qt in range(4):
                for c in range(NCH):
                    sl = slice(qt * 512, (qt + 1) * 512)
                    dsl = slice(PAD + qt * 512, PAD + (qt + 1) * 512)
                    nc.sync.dma_start(out=xpl[0][c][:, dsl],
                                      in_=xE_d[c * P:(c + 1) * P, sl])
                    nc.sync.dma_start(out=xpl[1][c][:, dsl],
                                      in_=xO_d[c * P:(c + 1) * P, sl])
            u_tiles[1] = load_u(1)

            def emit_conv(cidx, d, src, blk, conv):
                """One Winograd conv. src: planes pair-list. conv==1:
                snake -> h planes. conv==2: residual -> x planes, or
                y DMA if final block."""
                base = 6 * blk
                final = (blk == 2 and conv == 2)
                ut = u_tiles[cidx]
                if cidx + 2 <= 5:
                    u_tiles[cidx + 2] = load_u(cidx + 2)
                NL, units = geom(d)
                for (S, w) in units:
                    # ---- V build (shared across co) ----
                    vt = {}
                    for ci in range(NCH):
                        for a in range(4):
                            vt[(a, ci)] = pv.tile([P, 512], BF16,
                                                  tag=f"v{a}_{ci}",
                                                  name=f"v{a}_{ci}")
                        for (a, oo, st, cnt, s0, s1, op) in v_ops(d, S, w):
                            p0, c0, st0, _ = s0
                            p1_, c1, st1, _ = s1
                            alu1 = ALU.subtract if op == "sub" else ALU.add
                            nc.vector.scalar_tensor_tensor(
                                _ap_strided(vt[(a, ci)], oo, st, cnt, base=0),
                                _ap_strided(src[p0][ci], c0, st0, cnt),
                                1.0,
                                _ap_strided(src[p1_][ci], c1, st1, cnt),
                                ALU.mult, alu1)
                    for co in range(NCH):
                        ps = pps.tile([P, 2048], F32, tag="ps")
                        for a in range(4):
                            for ci in range(NCH):
                                nc.tensor.matmul(
                                    ps[:, a * 512:a * 512 + w],
                                    ut[a][ci][:, co * P:(co + 1) * P],
                                    vt[(a, ci)][:, 0:w],
                                    start=(ci == 0), stop=(ci == NCH - 1))
                        # ---- drain all 4 banks, scale by s ----
                        scol = base + (1 if conv == 1 else 5)
                        sap = pt[co][:, scol:scol + 1]
                        mq = pm.tile([P, 2048], F32, tag="mq")
                        ps3 = ps.rearrange("p (a x) -> p a x", x=512)[:, :, 0:w]
                        mq3 = mq[:, 0:4 * w].rearrange("p (a x) -> p a x", x=w)
                        nc.scalar.activation(mq3, ps3, AF.Identity, scale=sap)
                        m = [mq[:, a * w:(a + 1) * w] for a in range(4)]

                        if conv == 1:
                            bap = pt[co][:, base + 0:base + 1]
                            aap = pt[co][:, base + 2:base + 3]
                            iap = pt[co][:, base + 3:base + 4]
                            zt = psn.tile([P, 1024], BF16, tag="z")
                            t1 = ptm.tile([P, 512], F32, tag="t1")
                            nc.vector.scalar_tensor_tensor(
                                t1[:, 0:w], m[0], bap, m[1], ALU.add, ALU.add)
                            nc.vector.scalar_tensor_tensor(
                                zt[:, 0:w], t1[:, 0:w], 0.0, m[2],
                                ALU.add, ALU.add)
                            t2 = ptm.tile([P, 512], F32, tag="t2")
                            nc.vector.scalar_tensor_tensor(
                                t2[:, 0:w], m[1], 0.0, m[2],
                                ALU.add, ALU.subtract)
                            nc.vector.scalar_tensor_tensor(
                                zt[:, w:2 * w], t2[:, 0:w], bap, m[3],
                                ALU.add, ALU.subtract)
                            # snake: h = z + invb * sin(a z)^2
                            rt = psn.tile([P, 1024], F32, tag="r")
                            nc.vector.tensor_scalar(
                                out=rt[:, 0:2 * w], in0=zt[:, 0:2 * w],
                                scalar1=aap, scalar2=None, op0=ALU.mult)
                            ddt = psn.tile([P, 1024], F32, tag="dd")
                            if USE_MOD:
                                nc.vector.tensor_scalar(
                                    out=ddt[:, 0:2 * w], in0=rt[:, 0:2 * w],
                                    scalar1=1.0, scalar2=None, op0=ALU.mod)
                            else:
                                ri = psn.tile([P, 1024], I32, tag="ri")
                                nc.vector.tensor_copy(
                                    ri[:, 0:2 * w], rt[:, 0:2 * w])
                                nc.vector.scalar_tensor_tensor(
                                    ddt[:, 0:2 * w], rt[:, 0:2 * w], 1.0,
                                    ri[:, 0:2 * w], ALU.mult, ALU.subtract)
                            ust = psn.tile([P, 1024], BF16, tag="u")
                            nc.scalar.activation(ust[:, 0:2 * w],
                                                 ddt[:, 0:2 * w], AF.Sin,
                                                 scale=float(np.pi))
                            vst = psn.tile([P, 1024], BF16, tag="v")
                            nc.scalar.activation(vst[:, 0:2 * w],
                                                 ust[:, 0:2 * w], AF.Square)
                            for (hf, to, st, cnt, par, pc) in h_ops(d, S, w):
                                tsl = slice(hf * w + to,
                                            hf * w + to + (cnt - 1) * st + 1,
                                            st) if st > 1 else \
                                    slice(hf * w + to, hf * w + to + cnt)
                                nc.vector.scalar_tensor_tensor(
                                    _ap_strided(hpl[par][co], pc, st, cnt),
                                    vst[:, tsl], iap, zt[:, tsl],
                                    ALU.mult, ALU.add)
                        else:
                            bap = pt[co][:, base + 4:base + 5]
                            t1 = ptm.tile([P, 512], F32, tag="t1")
                            nc.vector.scalar_tensor_tensor(
                                t1[:, 0:w], m[0], bap, m[1], ALU.add, ALU.add)
                            g1 = ptm.tile([P, 512], F32, tag="g1")
                            nc.vector.scalar_tensor_tensor(
                                g1[:, 0:w], t1[:, 0:w], 0.0, m[2],
                                ALU.add, ALU.add)
                            t2 = ptm.tile([P, 512], F32, tag="t2")
                            nc.vector.scalar_tensor_tensor(
                                t2[:, 0:w], m[1], 0.0, m[2],
                                ALU.add, ALU.subtract)
                            g2 = ptm.tile([P, 512], F32, tag="g2")
                            nc.vector.scalar_tensor_tensor(
                                g2[:, 0:w], t2[:, 0:w], bap, m[3],
                                ALU.add, ALU.subtract)
                            if not final:
                                for hf, g in ((0, g1), (1, g2)):
                                    xs = _ap_strided(xpl[hf][co], S, 1, w)
                                    nc.vector.scalar_tensor_tensor(
                                        xs, g[:, 0:w], 0.0, xs,
                                        ALU.add, ALU.add)
                            else:
                                yt = py.tile([P, 1024], F32, tag="yt")
                                for hf, g in ((0, g1), (1, g2)):
                                    xs = _ap_strided(xpl[hf][co], S, 1, w)
                                    nc.vector.scalar_tensor_tensor(
                                        yt[:, hf:2 * w:2],
                                        g[:, 0:w], 0.0, xs,
                                        ALU.add, ALU.add)
                                nc.sync.dma_start(
                                    out=y_d[co * P:(co + 1) * P,
                                            2 * S:2 * S + 2 * w],
                                    in_=yt[:, 0:2 * w])

            for i, d in enumerate(DILATIONS):
                emit_conv(2 * i, d, xpl, i, 1)
                emit_conv(2 * i + 1, 1, hpl, i, 2)

    _split_sync_waits(nc)
    return nc


_NC = None


def _get_nc():
    global _NC
    if _NC is None:
        _NC = build_nc()
    return _NC


def _host_params(w1, b1, alpha, beta, w2, b2):
    """Ternarize weights, apply the Winograd weight transform G, fold
    snake params; matches the reference's jax-on-CPU float32 numerics."""
    import jax
    import jax.numpy as jnp

    cpu = jax.devices("cpu")[0]
    Gm = np.array([[1, 0, 0], [.5, .5, .5], [.5, -.5, .5], [0, 0, 1]],
                  np.float32)
    u = np.empty((6, 4, NCH, P, C), dtype=ml_dtypes.bfloat16)
    pp = np.zeros((NCH, P, NPARAM), dtype=np.float32)

    with jax.default_device(cpu):
        for i in range(3):
            for cv, (w, bias) in enumerate(((w1[i], b1[i]), (w2[i], b2[i]))):
                s = jnp.mean(jnp.abs(w))
                tern = np.asarray(
                    jnp.clip(jnp.round(w / (s + jnp.float32(EPS_Q))),
                             -1.0, 1.0), dtype=np.float32)
                U = np.einsum("ak,oik->aoi", Gm, tern)  # [4, Cout, Cin]
                u[2 * i + cv] = U.transpose(0, 2, 1).reshape(
                    4, NCH, P, C).astype(ml_dtypes.bfloat16)
                pp[:, :, 6 * i + (1 if cv == 0 else 5)] = np.float32(s)
                pp[:, :, 6 * i + (0 if cv == 0 else 4)] = \
                    np.asarray(bias, np.float32).reshape(NCH, P)
            a = np.asarray(jnp.exp(alpha[i]), dtype=np.float32)
            bsn = np.asarray(jnp.exp(beta[i]), dtype=np.float32)
            invb = np.asarray(
                jnp.float32(1.0) / (jnp.asarray(bsn) + jnp.float32(EPS_SNAKE)),
                dtype=np.float32)
            pp[:, :, 6 * i + 2] = (a / np.float32(np.pi)).reshape(NCH, P)
            pp[:, :, 6 * i + 3] = invb.reshape(NCH, P)
    return u, pp


def kernel(x, w1, b1, alpha, beta, w2, b2):
    global LAST_EXEC_NS, LAST_RESULT
    x = np.asarray(x, dtype=np.float32)
    w1 = np.asarray(w1, dtype=np.float32)
    b1 = np.asarray(b1, dtype=np.float32)
    alpha = np.asarray(alpha, dtype=np.float32)
    beta = np.asarray(beta, dtype=np.float32)
    w2 = np.asarray(w2, dtype=np.float32)
    b2 = np.asarray(b2, dtype=np.float32)

    u, pp = _host_params(w1, b1, alpha, beta, w2, b2)
    nc = _get_nc()

    in_maps = [
        {"xE": x[b, :, 0::2].astype(ml_dtypes.bfloat16),
         "xO": x[b, :, 1::2].astype(ml_dtypes.bfloat16),
         "u": u, "pp": pp}
        for b in range(B)
    ]
    res = run_bass_kernel_spmd(
        nc, in_maps, core_ids=list(range(B)), trace=TRACE)
    LAST_EXEC_NS = res.exec_time_ns
    LAST_RESULT = res

    out = np.stack([res.results[b]["y"] for b in range(B)], axis=0)
    return out.astype(np.float32)


# revision 7
# speedup vs baseline: 1.6737x; 1.6737x over previous
"""Trainium2 Bass kernel for nn_BitResidualBlock (dense_cnn).

Reference computation (per batch element, C=512 channels, T=4096):
    for d in (1, 3, 5):
        h = bitconv1d(x, w1, b1, dilation=d)     # ternary-quantized weights
        h = snake_beta(h, alpha, beta)           # x + sin(a*x)^2 / (b+eps)
        h = bitconv1d(h, w2, b2, dilation=1)
        x = x + h

Strategy:
  - Data-parallel over batch: 8 batch elements -> 8 NeuronCores, no
    collectives. Identical SPMD program, per-core input shard.
  - BitNet ternary quantization is done on HOST (it is a per-tensor
    scalar + ternarize): the ternary weights {-1,0,+1} are shipped as
    bf16 (exact), the scale s is applied in f32 on ScalarE.
  - Each conv = 12 accumulating 128x128x512 matmuls per output tile
    (4 ci chunks x 3 taps), bf16 operands, fp32 PSUM accumulate.
  - snake: z kept in f32; sin evaluated on ScalarE (LUT valid on
    [-pi, pi]) after range reduction mod pi using a f32->i32->f32
    round-trip (sin^2 is pi-periodic so any integer multiple works).
  - Residual x accumulates in the padded bf16 buffer (single
    scalar_tensor_tensor per tile); the final block writes f32 y
    tiles directly.
"""

import numpy as np
import ml_dtypes

import concourse.bass as bass
import concourse.mybir as mybir
import concourse.tile as tile
from concourse.vector_clock import ScopedClock
from concourse.bass_utils import run_bass_kernel_spmd

AF = mybir.ActivationFunctionType
ALU = mybir.AluOpType
F32 = mybir.dt.float32
I32 = mybir.dt.int32
BF16 = mybir.dt.bfloat16

B, C, T, K = 8, 512, 4096, 3
DILATIONS = (1, 3, 5)
EPS_Q = 1e-5
EPS_SNAKE = 1e-9

P = 128          # partitions
NCH = C // P     # 4 channel chunks
TT = 512         # time-tile (one PSUM bank of f32)
NT = T // TT     # 8 time tiles
PAD = 8          # zero pad each side of bf16 activation tiles
TPW = T + 2 * PAD
NPARAM = 21      # 7 param columns per block x 3 blocks

# Set by the test harness to profile; kernel() records exec time here.
TRACE = False
LAST_EXEC_NS = None
LAST_RESULT = None


class SplitDrainTileContext(tile.TileContext):
    """TileContext whose tail drain splits its sem waits across
    single-wait instructions.

    The walrus build in this environment rejects a Drain carrying more
    than a couple of sync waits ("Too many sync wait commands",
    CoreV3GenImpl.cpp setupSyncWait). Absorb the outstanding vector-clock
    waits with one single-wait nop per semaphore before draining.
    """

    def _drain_and_barrier(self, tick_clock, wait_clock):
        collector = self.nc.sync.nop(nofuse=True)
        wait_clock.add_sem_waits(
            collector.ins, ScopedClock({None: tick_clock.global_clock})
        )
        si = collector.ins.sync_info
        waits = list(si.on_wait) if si is not None else []
        if len(waits) > 1:
            collector.ins.sync_info = mybir.SyncInfo(
                on_wait=waits[:1], on_update=list(si.on_update)
            )
            for w in waits[1:]:
                extra = self.nc.sync.nop(nofuse=True)
                extra.ins.sync_info = mybir.SyncInfo(on_wait=[w], on_update=[])
        self.nc.sync.drain()
        self.nc.all_engine_barrier()
        assert self.sems is not None
        popped = self.nc._tile_sem_poison_stack.pop()
        assert popped is self._sem_poison
        self.nc.clear_and_free_semaphores(list(self.sems.allocated().values()))
        self.nc.all_engine_barrier()


def _split_sync_waits(nc, maxw=1):
    """Walrus in this environment encodes at most one sync wait per
    instruction ("Too many sync wait commands" otherwise). Move excess
    waits onto single-wait EventSemaphore instructions inserted just
    before the owner on the same engine (engines run their stream in
    block order, so the waits still gate the instruction)."""
    for bb in nc.main_func.blocks:
        out = []
        changed = False
        for ins in bb.instructions:
            si = getattr(ins, "sync_info", None)
            if si is not None and len(si.on_wait) > maxw:
                waits = list(si.on_wait)
                extra, keep = waits[:-maxw], waits[-maxw:]
                for w in extra:
                    ev = mybir.InstEventSemaphore(
                        name=nc.get_next_instruction_name(), ins=[], outs=[])
                    ev.engine = ins.engine
                    ev.sync_info = mybir.SyncInfo(on_wait=[w], on_update=[])
                    nc.register_instruction(ev, overwrite=True)
                    out.append(ev)
                ins.sync_info = mybir.SyncInfo(
                    on_wait=keep, on_update=list(si.on_update))
                changed = True
            out.append(ins)
        if changed:
            bb.instructions = out


def build_nc():
    nc = bass.Bass(target_bir_lowering=False)
    xb16_d = nc.dram_tensor("xb16", [C, T], BF16, kind="ExternalInput")
    wt_d = nc.dram_tensor("wt", [3, 2, NCH, P, K * NCH * P], BF16,
                          kind="ExternalInput")
    pp_d = nc.dram_tensor("pp", [NCH, P, NPARAM], F32, kind="ExternalInput")
    y_d = nc.dram_tensor("y", [C, T], F32, kind="ExternalOutput")

    with SplitDrainTileContext(nc) as tc:
        with (
            tc.tile_pool(name="persist", bufs=1) as p1,
            tc.tile_pool(name="wts", bufs=1) as pw,
            tc.tile_pool(name="t2", bufs=2) as p2,
            tc.tile_pool(name="t3", bufs=3) as p3,
            tc.tile_pool(name="tz", bufs=3) as pz,
            tc.tile_pool(name="yo", bufs=3) as py,
            tc.tile_pool(name="ps", bufs=6, space="PSUM") as pps,
        ):
            xb = [p1.tile([P, TPW], BF16, tag=f"xb{c}", name=f"xb{c}") for c in range(NCH)]
            hb = [p1.tile([P, TPW], BF16, tag=f"hb{c}", name=f"hb{c}") for c in range(NCH)]
            pt = [p1.tile([P, NPARAM], F32, tag=f"pt{c}", name=f"pt{c}") for c in range(NCH)]

            def alloc_w(i, conv):
                return [pw.tile([P, K * NCH * P], BF16,
                                tag=f"w{conv}_{c}", name=f"w{conv}_{i}_{c}")
                        for c in range(NCH)]

            def load_weights(i):
                w1t, w2t = alloc_w(i, 1), alloc_w(i, 2)
                for c in range(NCH):
                    nc.sync.dma_start(out=w1t[c], in_=wt_d[i, 0, c])
                for c in range(NCH):
                    nc.sync.dma_start(out=w2t[c], in_=wt_d[i, 1, c])
                return w1t, w2t

            for c in range(NCH):
                nc.sync.dma_start(out=pt[c], in_=pp_d[c])
                nc.vector.memset(xb[c][:, 0:PAD], 0.0)
                nc.vector.memset(xb[c][:, PAD + T:TPW], 0.0)
                nc.vector.memset(hb[c][:, 0:PAD], 0.0)
                nc.vector.memset(hb[c][:, PAD + T:TPW], 0.0)

            # All HWDGE DMAs drain through one FIFO queue at ~360 GB/s, so
            # the queue ORDER is the startup critical path. The first conv
            # matmuls need block-0 w1 (co=0 strip) + xb time-tiles 0..1;
            # then the rest of w1, w2, the rest of xb.
            w1t0 = alloc_w(0, 1)
            CW = K * P
            for c in range(NCH):
                nc.sync.dma_start(out=w1t0[c][:, 0:CW],
                                  in_=wt_d[0, 0, c][:, 0:CW])
            for jt in range(4):
                for c in range(NCH):
                    sl = slice(jt * TT, (jt + 1) * TT)
                    nc.sync.dma_start(
                        out=xb[c][:, PAD + jt * TT:PAD + (jt + 1) * TT],
                        in_=xb16_d[c * P:(c + 1) * P, sl])
            for c in range(NCH):
                nc.sync.dma_start(out=w1t0[c][:, CW:],
                                  in_=wt_d[0, 0, c][:, CW:])
            for jt in range(4, NT):
                for c in range(NCH):
                    sl = slice(jt * TT, (jt + 1) * TT)
                    nc.sync.dma_start(
                        out=xb[c][:, PAD + jt * TT:PAD + (jt + 1) * TT],
                        in_=xb16_d[c * P:(c + 1) * P, sl])
            w2t0 = alloc_w(0, 2)
            for c in range(NCH):
                nc.sync.dma_start(out=w2t0[c], in_=wt_d[0, 1, c])
            wcur = (w1t0, w2t0)

            for i in range(3):
                d = DILATIONS[i]
                base = i * 7
                w1t, w2t = wcur
                if i < 2:
                    wnext = load_weights(i + 1)

                # conv1 (dilation d) + snake -> hb (bf16, padded)
                for co in range(NCH):
                    b1ap = pt[co][:, base + 0:base + 1]
                    s1ap = pt[co][:, base + 1:base + 2]
                    raap = pt[co][:, base + 2:base + 3]
                    rbap = pt[co][:, base + 3:base + 4]
                    ibap = pt[co][:, base + 4:base + 5]
                    for jt in range(NT):
                        ps = pps.tile([P, TT], F32, tag="ps")
                        col0 = PAD + jt * TT
                        n = 0
                        for ci in range(NCH):
                            for k in range(K):
                                sh = (k - 1) * d
                                nc.tensor.matmul(
                                    ps,
                                    w1t[ci][:, (co * K + k) * P:
                                            (co * K + k + 1) * P],
                                    xb[ci][:, col0 + sh:col0 + sh + TT],
                                    start=(n == 0), stop=(n == 11),
                                )
                                n += 1
                        # z = s1*psum + b1 (the pre-activation, kept f32)
                        z = pz.tile([P, TT], F32, tag="z")
                        nc.scalar.activation(z, ps, AF.Identity,
                                             bias=b1ap, scale=s1ap)
                        # r = a*z/pi (folded: psum*(s1*a/pi) + b1*a/pi)
                        r = p3.tile([P, TT], F32, tag="r")
                        nc.scalar.activation(r, ps, AF.Identity,
                                             bias=rbap, scale=raap)
                        # range-reduce: dd = r - int(r)  (|dd| < 1)
                        ri = p2.tile([P, TT], I32, tag="ri")
                        nc.vector.tensor_copy(ri, r)
                        dd = p2.tile([P, TT], F32, tag="dd")
                        nc.vector.tensor_sub(dd, r, ri)
                        # u = sin(pi*dd) == +-sin(a*z);  u^2 is what we need
                        u = p3.tile([P, TT], F32, tag="u")
                        nc.scalar.activation(u, dd, AF.Sin,
                                             scale=float(np.pi))
                        v = p2.tile([P, TT], F32, tag="v")
                        nc.vector.tensor_mul(v, u, u)
                        # h = z + invb * u^2, cast to bf16 into padded hb
                        nc.vector.scalar_tensor_tensor(
                            hb[co][:, col0:col0 + TT], v, ibap, z,
                            ALU.mult, ALU.add,
                        )

                # conv2 (dilation 1) + residual add into xb (bf16) or y
                for co in range(NCH):
                    b2ap = pt[co][:, base + 5:base + 6]
                    s2ap = pt[co][:, base + 6:base + 7]
                    for jt in range(NT):
                        ps = pps.tile([P, TT], F32, tag="ps")
                        col0 = PAD + jt * TT
                        n = 0
                        for ci in range(NCH):
                            for k in range(K):
                                sh = k - 1
                                nc.tensor.matmul(
                                    ps,
                                    w2t[ci][:, (co * K + k) * P:
                                            (co * K + k + 1) * P],
                                    hb[ci][:, col0 + sh:col0 + sh + TT],
                                    start=(n == 0), stop=(n == 11),
                                )
                                n += 1
                        t = p3.tile([P, TT], F32, tag="t")
                        nc.scalar.activation(t, ps, AF.Identity,
                                             bias=b2ap, scale=s2ap)
                        xsl = xb[co][:, col0:col0 + TT]
                        if i < 2:
                            nc.vector.scalar_tensor_tensor(
                                xsl, t, 0.0, xsl, ALU.add, ALU.add)
                        else:
                            yt = py.tile([P, TT], F32, tag="yt")
                            nc.vector.scalar_tensor_tensor(
                                yt, t, 0.0, xsl, ALU.add, ALU.add)
                            nc.sync.dma_start(
                                out=y_d[co * P:(co + 1) * P,
                                        jt * TT:(jt + 1) * TT],
                                in_=yt)
                if i < 2:
                    wcur = wnext
    _split_sync_waits(nc)
    return nc


_NC = None


def _get_nc():
    global _NC
    if _NC is None:
        _NC = build_nc()
    return _NC


def _host_params(w1, b1, alpha, beta, w2, b2):
    """Ternarize weights and fold snake/scale params, matching the
    reference's jax-on-CPU float32 numerics."""
    import jax
    import jax.numpy as jnp

    cpu = jax.devices("cpu")[0]

    wt = np.empty((3, 2, NCH, P, K * NCH * P), dtype=ml_dtypes.bfloat16)
    pp = np.zeros((NCH, P, NPARAM), dtype=np.float32)
    pi = np.float32(np.pi)

    with jax.default_device(cpu):
        for i in range(3):
            svals = []
            for conv, w in ((0, w1[i]), (1, w2[i])):
                s = jnp.mean(jnp.abs(w))
                tern = jnp.clip(jnp.round(w / (s + EPS_Q)), -1.0, 1.0)
                svals.append(np.float32(s))
                tern = np.asarray(tern, dtype=np.float32)
                # [co, ci, k] -> [cich, ci_in, coch, k, co_in] (co-major
                # free dim so a single co chunk is one contiguous DMA)
                t5 = tern.reshape(NCH, P, NCH, P, K).transpose(2, 3, 0, 4, 1)
                wt[i, conv] = t5.reshape(NCH, P, K * NCH * P).astype(
                    ml_dtypes.bfloat16)
            s1, s2 = svals
            a = np.asarray(jnp.exp(alpha[i]), dtype=np.float32)
            bsn = np.asarray(jnp.exp(beta[i]), dtype=np.float32)
            invb = np.asarray(
                jnp.float32(1.0) / (jnp.asarray(bsn) + jnp.float32(EPS_SNAKE)),
                dtype=np.float32)
            base = i * 7
            pp[:, :, base + 0] = b1[i].reshape(NCH, P)
            pp[:, :, base + 1] = s1
            pp[:, :, base + 2] = (s1 * a / pi).reshape(NCH, P)
            pp[:, :, base + 3] = (b1[i] * a / pi).reshape(NCH, P)
            pp[:, :, base + 4] = invb.reshape(NCH, P)
            pp[:, :, base + 5] = b2[i].reshape(NCH, P)
            pp[:, :, base + 6] = s2
    return wt, pp


def kernel(x, w1, b1, alpha, beta, w2, b2):
    global LAST_EXEC_NS
    x = np.asarray(x, dtype=np.float32)
    w1 = np.asarray(w1, dtype=np.float32)
    b1 = np.asarray(b1, dtype=np.float32)
    alpha = np.asarray(alpha, dtype=np.float32)
    beta = np.asarray(beta, dtype=np.float32)
    w2 = np.asarray(w2, dtype=np.float32)
    b2 = np.asarray(b2, dtype=np.float32)

    wt, pp = _host_params(w1, b1, alpha, beta, w2, b2)
    nc = _get_nc()

    in_maps = [
        {"xb16": x[b].astype(ml_dtypes.bfloat16),
         "wt": wt, "pp": pp}
        for b in range(B)
    ]
    res = run_bass_kernel_spmd(
        nc, in_maps, core_ids=list(range(B)), trace=TRACE)
    LAST_EXEC_NS = res.exec_time_ns
    global LAST_RESULT
    LAST_RESULT = res

    out = np.stack([res.results[b]["y"] for b in range(B)], axis=0)
    return out.astype(np.float32)
